# revision 58
# baseline (speedup 1.0000x reference)
"""Locally-connected layer (unshared 3x3 conv, torch-unfold semantics) on 8 trn2 cores.

out[b,o,y,x] = sum_{c,i,j} weight[o, c*9+i*3+j, y*32+x] * xpad[b, c, y+i, x+j]

Sharding: spatial over L — core r owns image rows [4r, 4r+4) (128 pixels).

Design (fp8e3 weights x bf16 activations, weights-stationary, N=128 moving):
  * Weights are e3m4 fp8 on the wire, pre-scaled by 2^8; x is bf16 scaled by
    2^-8 (exact exponent shift), so the products need no on-chip descale.
    Mixed fp8xbf16 matmul is exact on the PE; measured end-to-end rel err
    ~1.35% (tolerance 2e-2).  fp8 halves weight DMA (9.4 -> 4.7 MB/core) —
    bf16 weights gate the matmul stream to ~99ns/MM, fp8 frees it to ~56ns.
  * SBUF slab T1 [128, (row, w, b)] = [slab | slab shifted +1 col].  The
    (w, b) free order makes every moving operand two contiguous 128B runs.
  * The 576-long contraction runs as 6 chunks per pixel pair; stationary =
    [K=128, 128] fp8 block covering BOTH pixels (cols m = 64*e + o); moving
    = t1r[:, row, x0:x0+2, :] (N = 2*64 = (pix, b)).  PSUM [128, (pix, b)]:
    only the e==pix half-partitions are read out.
      q0..q2: K=128 rows [c x (i=q,j=0) | c x (i=q,j=1)]
      s0..s2: K=128 rows [c x (s,2)     | 64 ZERO rows ]
    The s-chunks are zero-padded to K=128: every matmul is the same shape.
    A K=128<->K=64 size switch costs ~125ns extra on the PE (measured), so
    uniform K buys ~14us over the K-mixed version.  6 accumulating MMs per
    PSUM tile cost the same as independent MMs (measured 67 vs 67ns).
  * 384 MMs/core at ~56ns warm.  A ~24-MM warmup on zeroed data spans engine
    boot (~7.5us) to first-data (~12us) so HAM is at 8/8 for the real stream.
  * PSUM readout: 4 pairs per PSUM tile (1 bank), low half (e=0) via vector
    tensor_copy, high half via scalar activation-copy, amortizing the
    ~115-200ns PSUM access init.  Output bf16 [psum-partition, pair, b] in
    DRAM; host transposes to (B, O, H, W) fp32.
  * DMA: three generation paths used — x rows alternate scalar/gpsimd rings,
    weight groups ride sync (first half) and gpsimd (second half), outputs
    on scalar.  CRITICAL: a dma_start's semaphore waits (pool-slot reuse)
    block the ISSUING engine's sequencer, so weight triggers must never sit
    in the scalar stream where the readout copies live, and in-loop issue
    keeps each trigger's wait tight.  Total wire traffic: 3.34 (x bf16) +
    6.3 (w fp8, incl. s-chunk zero pad) + 1.05 (out bf16) MB per core.
"""

import numpy as np
import ml_dtypes

BF16 = ml_dtypes.bfloat16
F8E3 = ml_dtypes.float8_e3m4
WSCALE = 256.0  # w*2^8 into fp8e3; x*2^-8 in bf16 (exact) -> product unscaled

B, C, O, H, W, KS = 64, 64, 64, 32, 32, 3
L = H * W
NCORES = 8
RPC = H // NCORES            # image rows per core = 4
LC = RPC * W                 # pixels per core = 128
NP = LC // 2                 # pixel pairs per core = 64
HALO = RPC + 2               # 6 slab rows
WP = W + 2                   # padded width 34
PG = 8                       # pairs per weight DMA group
NG = NP // PG                # weight groups = 8

_CACHE = {}


def _build_nc():
    import concourse.bass as bass
    import concourse.bacc as bacc
    import concourse.tile as tile
    from concourse import mybir

    f32 = mybir.dt.float32
    bf16 = mybir.dt.bfloat16
    f8e3 = mybir.dt.float8e3
    nc = bacc.Bacc(
        "TRN2", target_bir_lowering=False, debug=False, num_devices=NCORES
    )
    x_d = nc.dram_tensor("x", [128, HALO, WP, B], bf16, kind="ExternalInput")
    wq_d = nc.dram_tensor("wq", [128, NP, 3, 128], f8e3, kind="ExternalInput")
    ws_d = nc.dram_tensor("ws", [128, NP, 3, 128], f8e3, kind="ExternalInput")
    o_d = nc.dram_tensor("out", [128, NP, B], bf16, kind="ExternalOutput")

    with tile.TileContext(nc) as tc:
        with (
            tc.tile_pool(name="x1", bufs=1) as x1pool,
            tc.tile_pool(name="wq", bufs=6) as wpool,
            tc.tile_pool(name="ws", bufs=6) as spool,
            tc.tile_pool(name="orow", bufs=6) as opool,
            tc.tile_pool(name="ps", bufs=7, space=bass.MemorySpace.PSUM) as pspool,
            tc.tile_pool(name="psw", bufs=1, space=bass.MemorySpace.PSUM) as pswpool,
        ):
            t1 = x1pool.tile([128, HALO * B * WP], bf16)
            t1r = t1[:].rearrange("p (r w b) -> p r w b", r=HALO, w=WP)
            # x rows land row-major, alternating scalar/gpsimd DGE paths so
            # consecutive rows transfer in parallel; row 0 lands first and
            # compute starts under the tail of the transfer.
            nc.scalar.dma_start(t1r[:, 0:1], x_d[:, 0:1])
            nc.gpsimd.dma_start(t1r[:, 1:2], x_d[:, 1:2])
            nc.scalar.dma_start(t1r[:, 2:3], x_d[:, 2:3])
            nc.gpsimd.dma_start(t1r[:, 3:4], x_d[:, 3:4])
            nc.scalar.dma_start(t1r[:, 4:5], x_d[:, 4:5])
            nc.gpsimd.dma_start(t1r[:, 5:6], x_d[:, 5:6])

            # PE warmup on zeroed data: fills the otherwise-idle window
            # between engine boot (~7.5us) and first data landing (~12us) so
            # the HAM clock gate is at 8/8 when the real stream begins.
            scr = x1pool.tile([128, 256], bf16)
            nc.vector.memzero(scr[:])
            psw = pswpool.tile([64, 256], f32)
            for _ in range(24):
                nc.tensor.matmul(psw[:], scr[:, 0:64], scr[:], start=True, stop=True)

            sizes = [4, 4, 8, 8, 8, 8, 8, 8, 4, 2, 2]
            bounds = np.cumsum([0] + sizes)
            gtiles = []
            for gi, cnt in enumerate(sizes):
                wt = wpool.tile([128, cnt, 3, 128], f8e3, tag="wq")
                st = spool.tile([128, cnt, 3, 128], f8e3, tag="ws")
                gtiles.append((cnt, wt, st))
                # first 5 groups occupy distinct pool slots (bufs=6): their
                # DMAs carry no reuse-waits, so fire them at engine boot on
                # the dedicated sync ring.
                if gi < 5:
                    g0, g1 = int(bounds[gi]), int(bounds[gi + 1])
                    nc.sync.dma_start(wt[:], wq_d[:, g0:g1])
                    nc.sync.dma_start(st[:], ws_d[:, g0:g1])
            for gi, (cnt, wt, st) in enumerate(gtiles):
                g0, g1 = int(bounds[gi]), int(bounds[gi + 1])
                if gi >= 5:
                    nc.gpsimd.dma_start(wt[:], wq_d[:, g0:g1])
                    nc.gpsimd.dma_start(st[:], ws_d[:, g0:g1])
                orow = opool.tile([128, cnt, B], bf16)
                for tt0 in range(0, cnt, 4):
                    kk = min(4, cnt - tt0)
                    ps = pspool.tile([128, 4, 2, B], f32)
                    # NOTE: each pair's 6 chunks must stay contiguous —
                    # PSUM has_written state is per (partition, bank), so an
                    # interleaved start=True from another pair flips earlier
                    # pairs' accumulation into overwrite (measured rel err
                    # 0.7 when q/s phases were split across the block).
                    for k in range(kk):
                        tt = tt0 + k
                        t = g0 + tt
                        y, x0 = (2 * t) // W, (2 * t) % W
                        for q in range(3):
                            nc.tensor.matmul(
                                ps[:, k], wt[:, tt, q, :],
                                t1r[:, y + q, x0 : x0 + 2, :],
                                start=(q == 0), stop=False,
                                skip_group_check=True,
                            )
                        for s in range(3):
                            nc.tensor.matmul(
                                ps[:, k], st[:, tt, s, :],
                                t1r[:, y + s, x0 + 2 : x0 + 4, :],
                                start=False, stop=(s == 2),
                                skip_group_check=True,
                            )
                    nc.vector.tensor_copy(
                        orow[0:64, tt0 : tt0 + kk, :], ps[0:64, 0:kk, 0, :]
                    )
                    nc.scalar.copy(
                        orow[64:128, tt0 : tt0 + kk, :], ps[64:128, 0:kk, 1, :]
                    )
                nc.scalar.dma_start(o_d[:, g0:g1, :], orow[:])
    nc.compile()
    return nc


def _get_nc():
    if "nc" not in _CACHE:
        _CACHE["nc"] = _build_nc()
    return _CACHE["nc"]


def _pack_x(x):
    """Per core: [128, HALO, WP, B] bf16 = [slab | slab shifted +1 col],
    row-major so row blocks stream independently; scaled by 2^-8 (exact)
    to cancel the 2^8 pre-scale baked into the fp8 weights."""
    xpad = np.pad(x, ((0, 0), (0, 0), (1, 1), (1, 1)))
    xpad = np.ascontiguousarray(xpad.transpose(1, 0, 2, 3))  # [C, B, 34, 34]
    outs = []
    for r in range(NCORES):
        slab = xpad[:, :, RPC * r : RPC * r + HALO, :]       # [C, B, 6, 34]
        slab = slab.transpose(0, 2, 3, 1)                    # [C, 6, 34, B]
        up = np.zeros_like(slab)
        up[:, :, : WP - 1, :] = slab[:, :, 1:, :]
        t1 = (np.concatenate([slab, up], axis=0) * (1.0 / WSCALE)).astype(BF16)
        outs.append(np.ascontiguousarray(t1))
    return outs


def _pack_w(weight):
    """Chunked-contraction weight blobs, already in SBUF layout.

    wq: [core, NG, p=(j, c), tt, q, m=(e, o)]   (pair chunks, shifts (q, j))
    ws: [core, NG, c, tt, s, m=(e, o)]          (singles, shifts (s, 2))
    """
    w5 = weight.reshape(O, C, KS, KS, L)
    low = np.stack([w5[:, :, 0, 0], w5[:, :, 1, 0], w5[:, :, 2, 0]], axis=0)
    up = np.stack([w5[:, :, 0, 1], w5[:, :, 1, 1], w5[:, :, 2, 1]], axis=0)
    wq = np.stack([low, up], axis=1)          # [q, j, O, C, L]
    wq = wq.reshape(3, 2, O, C, NCORES, NP, 2)
    # -> [core, j, c, t, q, e, o]
    wq = wq.transpose(4, 1, 3, 5, 0, 6, 2)
    wq = np.clip(np.ascontiguousarray(wq) * WSCALE, -15.5, 15.5)
    wq = wq.astype(F8E3).reshape(NCORES, 128, NP, 3, 128)

    ws = np.stack([w5[:, :, 0, 2], w5[:, :, 1, 2], w5[:, :, 2, 2]], axis=0)
    ws = ws.reshape(3, O, C, NCORES, NP, 2)
    ws = ws.transpose(3, 2, 4, 0, 5, 1)       # [core, c, t, s, e, o]
    ws = np.clip(np.ascontiguousarray(ws) * WSCALE, -15.5, 15.5)
    ws = ws.astype(F8E3).reshape(NCORES, 64, NP, 3, 128)
    # pad contraction rows 64:128 with zero weights: keeps every matmul at
    # K=128 (K-size switches cost ~125ns each on the PE)
    wsp = np.zeros((NCORES, 128, NP, 3, 128), F8E3)
    wsp[:, 0:64] = ws
    return wq, wsp


def kernel(x, weight, bias, _trace=False, _trace_kwargs=None):
    from concourse.bass_utils import run_bass_kernel_spmd

    x = np.asarray(x, dtype=np.float32)
    weight = np.asarray(weight, dtype=np.float32)
    bias = np.asarray(bias, dtype=np.float32)

    nc = _get_nc()
    xs = _pack_x(x)
    wq, ws = _pack_w(weight)
    in_maps = [
        {"x": xs[r], "wq": wq[r], "ws": ws[r]} for r in range(NCORES)
    ]
    res = run_bass_kernel_spmd(
        nc, in_maps, list(range(NCORES)),
        trace=_trace, **(_trace_kwargs or {}),
    )
    # out[r]: [p=(e,o), t, b] bf16 -> [b, o, l=128r+2t+e]
    parts = []
    for r in range(NCORES):
        arr = res.results[r]["out"].astype(np.float32)
        arr = arr.reshape(2, O, NP, B).transpose(3, 1, 2, 0)  # [b, o, t, e]
        parts.append(arr.reshape(B, O, LC))
    out = np.concatenate(parts, axis=2).reshape(B, O, H, W)
    if np.any(bias):
        out = out + bias.reshape(1, O, H, W)
    if _trace:
        _CACHE["last_result"] = res
    return np.ascontiguousarray(out.astype(np.float32))



# revision 59
# speedup vs baseline: 1.0452x; 1.0452x over previous
"""Locally-connected layer (unshared 3x3 conv, torch-unfold semantics) on 8 trn2 cores.

out[b,o,y,x] = sum_{c,i,j} weight[o, c*9+i*3+j, y*32+x] * xpad[b, c, y+i, x+j]

Sharding: spatial over L — core r owns image rows [4r, 4r+4) (128 pixels).

Design (fp8e3 weights x bf16 activations, weights-stationary, N=128 moving):
  * Weights are e3m4 fp8 on the wire, pre-scaled by 2^8; x is bf16 scaled by
    2^-8 (exact exponent shift), so the products need no on-chip descale.
    Mixed fp8xbf16 matmul is exact on the PE; measured end-to-end rel err
    ~1.35% (tolerance 2e-2).  fp8 halves weight DMA (9.4 -> 4.7 MB/core) —
    bf16 weights gate the matmul stream to ~99ns/MM, fp8 frees it to ~56ns.
  * SBUF slab T1 [128, (row, w, b)] = [slab | slab shifted +1 col].  The
    (w, b) free order makes every moving operand two contiguous 128B runs.
  * The 576-long contraction runs as 6 chunks per pixel pair; stationary =
    [K=128, 128] fp8 block covering BOTH pixels (cols m = 64*e + o); moving
    = t1r[:, row, x0:x0+2, :] (N = 2*64 = (pix, b)).  PSUM [128, (pix, b)]:
    only the e==pix half-partitions are read out.
      q0..q2: K=128 rows [c x (i=q,j=0) | c x (i=q,j=1)]
      s0..s2: K=128 rows [c x (s,2)     | 64 ZERO rows ]
    The s-chunks are zero-padded to K=128: every matmul is the same shape.
    A K=128<->K=64 size switch costs ~125ns extra on the PE (measured), so
    uniform K buys ~14us over the K-mixed version.  6 accumulating MMs per
    PSUM tile cost the same as independent MMs (measured 67 vs 67ns).
  * 384 MMs/core at ~56ns warm.  A ~24-MM warmup on zeroed data spans engine
    boot (~7.5us) to first-data (~12us) so HAM is at 8/8 for the real stream.
  * PSUM readout: 4 pairs per PSUM tile (1 bank), low half (e=0) via vector
    tensor_copy, high half via scalar activation-copy, amortizing the
    ~115-200ns PSUM access init.  Output bf16 [psum-partition, pair, b] in
    DRAM; host transposes to (B, O, H, W) fp32.
  * DMA: three generation paths used — x rows alternate scalar/gpsimd rings,
    weight groups ride sync (first half) and gpsimd (second half), outputs
    on scalar.  CRITICAL: a dma_start's semaphore waits (pool-slot reuse)
    block the ISSUING engine's sequencer, so weight triggers must never sit
    in the scalar stream where the readout copies live, and in-loop issue
    keeps each trigger's wait tight.  Total wire traffic: 3.34 (x bf16) +
    6.3 (w fp8, incl. s-chunk zero pad) + 1.05 (out bf16) MB per core.
"""

import numpy as np
import ml_dtypes

BF16 = ml_dtypes.bfloat16
F8E3 = ml_dtypes.float8_e3m4
WSCALE = 256.0  # w*2^8 into fp8e3; x*2^-8 in bf16 (exact) -> product unscaled

B, C, O, H, W, KS = 64, 64, 64, 32, 32, 3
L = H * W
NCORES = 8
RPC = H // NCORES            # image rows per core = 4
LC = RPC * W                 # pixels per core = 128
NP = LC // 2                 # pixel pairs per core = 64
HALO = RPC + 2               # 6 slab rows
WP = W + 2                   # padded width 34
PG = 8                       # pairs per weight DMA group
NG = NP // PG                # weight groups = 8

_CACHE = {}


def _build_nc():
    import concourse.bass as bass
    import concourse.bacc as bacc
    import concourse.tile as tile
    from concourse import mybir

    f32 = mybir.dt.float32
    bf16 = mybir.dt.bfloat16
    f8e3 = mybir.dt.float8e3
    nc = bacc.Bacc(
        "TRN2", target_bir_lowering=False, debug=False, num_devices=NCORES
    )
    x_d = nc.dram_tensor("x", [128, HALO, WP, B], bf16, kind="ExternalInput")
    wq_d = nc.dram_tensor("wq", [128, NP, 3, 128], f8e3, kind="ExternalInput")
    ws_d = nc.dram_tensor("ws", [128, NP, 3, 128], f8e3, kind="ExternalInput")
    o_d = nc.dram_tensor("out", [128, NP, B], bf16, kind="ExternalOutput")

    with tile.TileContext(nc) as tc:
        with (
            tc.tile_pool(name="x1", bufs=1) as x1pool,
            tc.tile_pool(name="wq", bufs=6) as wpool,
            tc.tile_pool(name="ws", bufs=6) as spool,
            tc.tile_pool(name="orow", bufs=3) as opool,
            tc.tile_pool(name="ps", bufs=7, space=bass.MemorySpace.PSUM) as pspool,
            tc.tile_pool(name="psw", bufs=1, space=bass.MemorySpace.PSUM) as pswpool,
        ):
            t1 = x1pool.tile([128, HALO * B * WP], bf16)
            t1r = t1[:].rearrange("p (r w b) -> p r w b", r=HALO, w=WP)
            # x rows land row-major, alternating scalar/gpsimd DGE paths so
            # consecutive rows transfer in parallel; row 0 lands first and
            # compute starts under the tail of the transfer.
            nc.scalar.dma_start(t1r[:, 0:1], x_d[:, 0:1])
            nc.gpsimd.dma_start(t1r[:, 1:2], x_d[:, 1:2])
            nc.scalar.dma_start(t1r[:, 2:3], x_d[:, 2:3])
            nc.gpsimd.dma_start(t1r[:, 3:4], x_d[:, 3:4])
            nc.scalar.dma_start(t1r[:, 4:5], x_d[:, 4:5])
            nc.gpsimd.dma_start(t1r[:, 5:6], x_d[:, 5:6])

            # PE warmup on zeroed data: fills the otherwise-idle window
            # between engine boot (~7.5us) and first data landing (~12us) so
            # the HAM clock gate is at 8/8 when the real stream begins.
            scr = x1pool.tile([128, 256], bf16)
            nc.vector.memzero(scr[:])
            psw = pswpool.tile([64, 256], f32)
            for _ in range(24):
                nc.tensor.matmul(psw[:], scr[:, 0:64], scr[:], start=True, stop=True)

            sizes = [4, 4, 8, 8, 8, 8, 8, 8, 4, 2, 2]
            bounds = np.cumsum([0] + sizes)
            gtiles = []
            for gi, cnt in enumerate(sizes):
                wt = wpool.tile([128, cnt, 3, 128], f8e3, tag="wq")
                st = spool.tile([128, cnt, 3, 128], f8e3, tag="ws")
                gtiles.append((cnt, wt, st))
                # first 5 groups occupy distinct pool slots (bufs=6): their
                # DMAs carry no reuse-waits, so fire them at engine boot on
                # the dedicated sync ring.
                if gi < 5:
                    g0, g1 = int(bounds[gi]), int(bounds[gi + 1])
                    nc.sync.dma_start(wt[:], wq_d[:, g0:g1])
                    nc.sync.dma_start(st[:], ws_d[:, g0:g1])
            for gi, (cnt, wt, st) in enumerate(gtiles):
                g0, g1 = int(bounds[gi]), int(bounds[gi + 1])
                if gi >= 5:
                    nc.gpsimd.dma_start(wt[:], wq_d[:, g0:g1])
                    nc.gpsimd.dma_start(st[:], ws_d[:, g0:g1])
                orow = opool.tile([128, cnt, B], bf16)
                for tt0 in range(0, cnt, 4):
                    kk = min(4, cnt - tt0)
                    ps = pspool.tile([128, 4, 2, B], f32)
                    # NOTE: each pair's 6 chunks must stay contiguous —
                    # PSUM has_written state is per (partition, bank), so an
                    # interleaved start=True from another pair flips earlier
                    # pairs' accumulation into overwrite (measured rel err
                    # 0.7 when q/s phases were split across the block).
                    for k in range(kk):
                        tt = tt0 + k
                        t = g0 + tt
                        y, x0 = (2 * t) // W, (2 * t) % W
                        for q in range(3):
                            nc.tensor.matmul(
                                ps[:, k], wt[:, tt, q, :],
                                t1r[:, y + q, x0 : x0 + 2, :],
                                start=(q == 0), stop=False,
                                skip_group_check=True,
                            )
                        for s in range(3):
                            nc.tensor.matmul(
                                ps[:, k], st[:, tt, s, :],
                                t1r[:, y + s, x0 + 2 : x0 + 4, :],
                                start=False, stop=(s == 2),
                                skip_group_check=True,
                            )
                    nc.vector.tensor_copy(
                        orow[0:64, tt0 : tt0 + kk, :], ps[0:64, 0:kk, 0, :]
                    )
                    nc.scalar.copy(
                        orow[64:128, tt0 : tt0 + kk, :], ps[64:128, 0:kk, 1, :]
                    )
                nc.scalar.dma_start(o_d[:, g0:g1, :], orow[:])
    nc.compile()
    return nc


def _get_nc():
    if "nc" not in _CACHE:
        _CACHE["nc"] = _build_nc()
    return _CACHE["nc"]


def _pack_x(x):
    """Per core: [128, HALO, WP, B] bf16 = [slab | slab shifted +1 col],
    row-major so row blocks stream independently; scaled by 2^-8 (exact)
    to cancel the 2^8 pre-scale baked into the fp8 weights."""
    xpad = np.pad(x, ((0, 0), (0, 0), (1, 1), (1, 1)))
    xpad = np.ascontiguousarray(xpad.transpose(1, 0, 2, 3))  # [C, B, 34, 34]
    outs = []
    for r in range(NCORES):
        slab = xpad[:, :, RPC * r : RPC * r + HALO, :]       # [C, B, 6, 34]
        slab = slab.transpose(0, 2, 3, 1)                    # [C, 6, 34, B]
        up = np.zeros_like(slab)
        up[:, :, : WP - 1, :] = slab[:, :, 1:, :]
        t1 = (np.concatenate([slab, up], axis=0) * (1.0 / WSCALE)).astype(BF16)
        outs.append(np.ascontiguousarray(t1))
    return outs


def _pack_w(weight):
    """Chunked-contraction weight blobs, already in SBUF layout.

    wq: [core, NG, p=(j, c), tt, q, m=(e, o)]   (pair chunks, shifts (q, j))
    ws: [core, NG, c, tt, s, m=(e, o)]          (singles, shifts (s, 2))
    """
    w5 = weight.reshape(O, C, KS, KS, L)
    low = np.stack([w5[:, :, 0, 0], w5[:, :, 1, 0], w5[:, :, 2, 0]], axis=0)
    up = np.stack([w5[:, :, 0, 1], w5[:, :, 1, 1], w5[:, :, 2, 1]], axis=0)
    wq = np.stack([low, up], axis=1)          # [q, j, O, C, L]
    wq = wq.reshape(3, 2, O, C, NCORES, NP, 2)
    # -> [core, j, c, t, q, e, o]
    wq = wq.transpose(4, 1, 3, 5, 0, 6, 2)
    wq = np.clip(np.ascontiguousarray(wq) * WSCALE, -15.5, 15.5)
    wq = wq.astype(F8E3).reshape(NCORES, 128, NP, 3, 128)

    ws = np.stack([w5[:, :, 0, 2], w5[:, :, 1, 2], w5[:, :, 2, 2]], axis=0)
    ws = ws.reshape(3, O, C, NCORES, NP, 2)
    ws = ws.transpose(3, 2, 4, 0, 5, 1)       # [core, c, t, s, e, o]
    ws = np.clip(np.ascontiguousarray(ws) * WSCALE, -15.5, 15.5)
    ws = ws.astype(F8E3).reshape(NCORES, 64, NP, 3, 128)
    # pad contraction rows 64:128 with zero weights: keeps every matmul at
    # K=128 (K-size switches cost ~125ns each on the PE)
    wsp = np.zeros((NCORES, 128, NP, 3, 128), F8E3)
    wsp[:, 0:64] = ws
    return wq, wsp


def kernel(x, weight, bias, _trace=False, _trace_kwargs=None):
    from concourse.bass_utils import run_bass_kernel_spmd

    x = np.asarray(x, dtype=np.float32)
    weight = np.asarray(weight, dtype=np.float32)
    bias = np.asarray(bias, dtype=np.float32)

    nc = _get_nc()
    xs = _pack_x(x)
    wq, ws = _pack_w(weight)
    in_maps = [
        {"x": xs[r], "wq": wq[r], "ws": ws[r]} for r in range(NCORES)
    ]
    res = run_bass_kernel_spmd(
        nc, in_maps, list(range(NCORES)),
        trace=_trace, **(_trace_kwargs or {}),
    )
    # out[r]: [p=(e,o), t, b] bf16 -> [b, o, l=128r+2t+e]
    parts = []
    for r in range(NCORES):
        arr = res.results[r]["out"].astype(np.float32)
        arr = arr.reshape(2, O, NP, B).transpose(3, 1, 2, 0)  # [b, o, t, e]
        parts.append(arr.reshape(B, O, LC))
    out = np.concatenate(parts, axis=2).reshape(B, O, H, W)
    if np.any(bias):
        out = out + bias.reshape(1, O, H, W)
    if _trace:
        _CACHE["last_result"] = res
    return np.ascontiguousarray(out.astype(np.float32))

